# revision 1
# baseline (speedup 1.0000x reference)
"""Trainium2 Bass kernel for nn_CedrDrmmRanker (histogram_binning).

Computation (per layer l, batch b over hidden_states [13,16,512,768] f32):
  sim[q,d] = cos(x_q, x_d) for q in first 20 tokens, d in remaining 492
  hist     = 11-bin histogram of sim over [-1,1]
  hfeat    = hist @ W_hist.T + b_hist
  out[b]   = concat(cls, hfeat-all-layers) @ W_comb.T + b_comb

Device strategy (pure data parallel, batch sharded 2-per-core over 8 cores):
  Per (l,b) pair: cast-DMA fp32->bf16, PE transpose via identity matmuls,
  PE Gram matmul q x d, DVE sum-of-squares norms, boundary counts via
  fused is_ge+accumulate.  Device emits only per-(pair,q-row) >=boundary
  counts; the tiny histogram/linear algebra runs on host in fp32.
"""

import os
import sys

import numpy as np

for _p in ("/opt/trn_rl_repo",):
    if os.path.isdir(_p) and _p not in sys.path:
        sys.path.append(_p)

# ---- problem constants (hardcoded; kernel.py must be self-contained) ----
L = 13          # layers
B = 16          # global batch
S = 512         # sequence
H = 768         # hidden
NQ = 20         # query tokens
ND = S - NQ     # 492 doc tokens
N_BINS = 11
N_CORES = 8
BC = B // N_CORES          # 2 batches per core
PAIRS = L * BC             # 26 (layer-major: p = l*BC + b)
GSIZE = 4                  # pairs per count-group (32-partition slots)
NGROUPS = (PAIRS + GSIZE - 1) // GSIZE   # 7
NB = 10                    # interior boundaries b1..b10 counted on device
SCH = S // 128             # 4 S-chunks
HCH = H // 128             # 6 H-chunks

_BOUNDS = np.linspace(-1.0, 1.0, N_BINS + 1).astype(np.float32)  # 12 boundaries


def _build_nc(npairs=PAIRS, num_devices=N_CORES, nreps=1):
    import concourse.bass as bass
    import concourse.tile as tile
    from concourse import bacc, mybir
    from concourse.masks import make_identity
    from contextlib import ExitStack

    f32 = mybir.dt.float32
    bf16 = mybir.dt.bfloat16
    ngroups = (npairs + GSIZE - 1) // GSIZE

    nc = bacc.Bacc(
        "TRN2",
        target_bir_lowering=False,
        debug=False,
        num_devices=num_devices,
    )
    hs = nc.dram_tensor("hs", [L, BC, S, H], f32, kind="ExternalInput").ap()
    counts = nc.dram_tensor(
        "counts", [NGROUPS, 128, NB], f32, kind="ExternalOutput"
    ).ap()

    with tile.TileContext(nc) as tc, ExitStack() as ctx:
        consts = ctx.enter_context(tc.tile_pool(name="consts", bufs=1))
        xpool = ctx.enter_context(tc.tile_pool(name="x", bufs=5))
        xtpool = ctx.enter_context(tc.tile_pool(name="xt", bufs=4))
        sqpool = ctx.enter_context(tc.tile_pool(name="sq", bufs=4))
        npool = ctx.enter_context(tc.tile_pool(name="nrm", bufs=4))
        gpool = ctx.enter_context(tc.tile_pool(name="grp", bufs=3))
        psA = ctx.enter_context(tc.tile_pool(name="psA", bufs=3, space="PSUM"))
        psB = ctx.enter_context(tc.tile_pool(name="psB", bufs=4, space="PSUM"))
        psC = ctx.enter_context(tc.tile_pool(name="psC", bufs=1, space="PSUM"))

        ident_bf = consts.tile([128, 128], bf16, tag="identb")
        make_identity(nc, ident_bf[:])
        ident_f32 = consts.tile([128, 128], f32, tag="identf")
        make_identity(nc, ident_f32[:])

        mult = mybir.AluOpType.mult
        add = mybir.AluOpType.add
        bypass = mybir.AluOpType.bypass
        is_ge = mybir.AluOpType.is_ge

        rep_ctx = tc.For_i(0, nreps, 1) if nreps > 1 else None
        if rep_ctx is not None:
            ctx.enter_context(rep_ctx)

        for g in range(ngroups):
            gp = min(GSIZE, npairs - g * GSIZE)  # pairs in this group
            simgrp = gpool.tile([128, ND], f32, tag="sim")
            dmi = gpool.tile([128, S], f32, tag="dmi")
            # rows between pair slots are never written; park them below -1 so
            # every is_ge boundary count sees 0 there (host ignores them).
            nc.vector.memset(simgrp[:], -2.0)

            for i in range(gp):
                p = g * GSIZE + i
                l, b = divmod(p, BC)
                r0 = 32 * i  # partition row base for this pair
                # own PSUM bank per pair (padded to one 2 KiB bank) so the
                # sim read doesn't serialize against other pairs' matmuls
                dots_full = psB.tile([128, 512], f32, tag="dots")
                dots_ps = dots_full[:, :ND]

                # 1) load + cast fp32 -> bf16   [128, 4, 768]
                xb = xpool.tile([128, SCH, H], bf16, tag="xb")
                nc.gpsimd.dma_start(
                    xb[:], hs[l, b].rearrange("(t p) h -> p t h", p=128)
                )

                # 2) token norms^2 via fused square+reduce  -> n2 [128, 4]
                #    split across ACT (Square+accum) and DVE (stt+accum)
                n2 = npool.tile([128, SCH], f32, tag="n2")
                for t in range(SCH):
                    sq = sqpool.tile([128, H], bf16, tag="sq")
                    if t < 2:
                        nc.scalar.activation(
                            out=sq[:],
                            in_=xb[:, t],
                            func=mybir.ActivationFunctionType.Square,
                            accum_out=n2[:, t : t + 1],
                        )
                    else:
                        nc.vector.scalar_tensor_tensor(
                            out=sq[:],
                            in0=xb[:, t],
                            scalar=0.0,
                            in1=xb[:, t],
                            op0=bypass,
                            op1=mult,
                            accum_out=n2[:, t : t + 1],
                        )

                # 3) inv norms (column layout)
                nrm = npool.tile([128, SCH], f32, tag="nrmc")
                nc.scalar.sqrt(nrm[:], n2[:])
                inv_col = npool.tile([128, SCH], f32, tag="invc")
                nc.vector.reciprocal(inv_col[:], nrm[:])

                # 4) pre-scale the 20 q rows (S-chunk 0) by their inv norm so
                #    the Gram matmul emits q-normalized dots.  (ACT: DVE is
                #    the busier engine.)
                nc.scalar.activation(
                    out=xb[0:NQ, 0],
                    in_=xb[0:NQ, 0],
                    func=mybir.ActivationFunctionType.Copy,
                    scale=inv_col[0:NQ, 0:1],
                )

                # 5) inv norms as rows: PE transpose [128,4] -> [4,128]
                invT_full = psC.tile([SCH, 512], f32, tag="invT")
                invT = invT_full[:, :128]
                nc.tensor.transpose(invT, inv_col[:], ident_f32[:])
                inv_row = npool.tile([SCH, 128], f32, tag="invr")
                nc.vector.tensor_copy(out=inv_row[:], in_=invT[:])

                # 6) broadcast inv_row into this pair's rows of dmi [20,512]
                for t in range(SCH):
                    nc.sync.dma_start(
                        dmi[r0 : r0 + NQ, t * 128 : (t + 1) * 128],
                        inv_row[t : t + 1, :]
                        .unsqueeze(1)
                        .broadcast_to((1, NQ, 128)),
                    )

                # 7) transpose X via PE identity matmuls -> XT [128, 6, 512]
                xt = xtpool.tile([128, HCH, S], bf16, tag="xt")
                for j in range(3):  # 2 H-chunks per PSUM bank tile
                    xtps = psA.tile([128, 2 * S], bf16, tag="xtps")
                    for u in range(2):
                        h = 2 * j + u
                        for t in range(SCH):
                            nc.tensor.transpose(
                                xtps[:, u * S + t * 128 : u * S + (t + 1) * 128],
                                xb[:, t, h * 128 : (h + 1) * 128],
                                ident_bf[:],
                            )
                    # PSUM -> SBUF copy (vector for 2 of 3, scalar for 1)
                    xt_dst = xt[:, 2 * j : 2 * j + 2, :].rearrange("p a b -> p (a b)")
                    if j == 2:
                        nc.scalar.copy(out=xt_dst, in_=xtps[:])
                    else:
                        nc.vector.tensor_copy(out=xt_dst, in_=xtps[:])

                # 8) dots = qT.T @ dT  accumulated over 6 H-chunks
                for h in range(HCH):
                    nc.tensor.matmul(
                        dots_ps[r0 : r0 + NQ, :],
                        lhsT=xt[:, h, 0:NQ],
                        rhs=xt[:, h, NQ:S],
                        start=(h == 0),
                        stop=(h == HCH - 1),
                        tile_position=(0, r0),
                    )

                # 9) sim = dots * inv_d  (q already normalized in the matmul)
                nc.vector.scalar_tensor_tensor(
                    out=simgrp[r0 : r0 + NQ, :],
                    in0=dots_ps[r0 : r0 + NQ, :],
                    scalar=0.0,
                    in1=dmi[r0 : r0 + NQ, NQ:S],
                    op0=bypass,
                    op1=mult,
                )

            # 10) boundary counts: cnt[:, k] = sum_d (sim >= b_k)
            cntg = gpool.tile([128, NB], f32, tag="cnt")
            for k in range(NB):
                csc = sqpool.tile([128, ND], bf16, tag="csc")
                nc.vector.tensor_scalar(
                    out=csc[:],
                    in0=simgrp[:],
                    scalar1=float(_BOUNDS[k + 1]),
                    scalar2=None,
                    op0=is_ge,
                    op1=add,
                    accum_out=cntg[:, k : k + 1],
                )
            nc.sync.dma_start(counts[g], cntg[:])

    nc.compile()
    return nc


_NC_CACHE = None


def _get_nc():
    global _NC_CACHE
    if _NC_CACHE is None:
        _NC_CACHE = _build_nc()
    return _NC_CACHE


def _postprocess(counts_per_core, hidden_states, W_hist, b_hist, W_comb, b_comb):
    """counts_per_core: list of 8 arrays [NGROUPS, 128, NB]."""
    hs = np.asarray(hidden_states, dtype=np.float32)
    W_hist = np.asarray(W_hist, np.float32)
    b_hist = np.asarray(b_hist, np.float32)
    W_comb = np.asarray(W_comb, np.float32)
    b_comb = np.asarray(b_comb, np.float32)

    # N_ge counts per (core, pair, boundary)
    hist = np.zeros((L, B, N_BINS), np.float32)
    total = float(NQ * ND)
    for c in range(N_CORES):
        cc = counts_per_core[c]  # [NGROUPS, 128, NB]
        for p in range(PAIRS):
            g, i = divmod(p, GSIZE)
            l, bl = divmod(p, BC)
            n_ge = cc[g, 32 * i : 32 * i + NQ, :].sum(axis=0)  # [NB]
            n_full = np.empty(N_BINS + 1, np.float64)
            n_full[0] = total
            n_full[1 : NB + 1] = n_ge
            n_full[N_BINS] = 0.0
            hist[l, c * BC + bl] = (n_full[:-1] - n_full[1:]) / total

    # histogram features for the 14 "all_layers" (layer 0 duplicated)
    hist14 = np.concatenate([hist[:1], hist], axis=0)  # [14, B, 11]
    hfeat = hist14 @ W_hist.T + b_hist  # [14, B, 5]
    histogram_features = np.transpose(hfeat, (1, 0, 2)).reshape(B, -1)  # [B, 70]

    cls_output = hs[-1][:, 0, :]  # [B, H]
    combined = np.concatenate([cls_output, histogram_features], axis=-1)
    return (combined @ W_comb.T + b_comb).astype(np.float32)  # [B, 1]


def kernel(hidden_states, W_hist, b_hist, W_comb, b_comb):
    from concourse.bass_utils import run_bass_kernel_spmd

    nc = _get_nc()
    hs = np.ascontiguousarray(np.asarray(hidden_states, dtype=np.float32))
    in_maps = [
        {"hs": np.ascontiguousarray(hs[:, c * BC : (c + 1) * BC])}
        for c in range(N_CORES)
    ]
    res = run_bass_kernel_spmd(nc, in_maps, core_ids=list(range(N_CORES)))
    counts_per_core = [np.asarray(res.results[c]["counts"]) for c in range(N_CORES)]
    return _postprocess(
        counts_per_core, hidden_states, W_hist, b_hist, W_comb, b_comb
    )

